# revision 43
# baseline (speedup 1.0000x reference)
"""Trainium2 Bass kernel for the quantized LM-head (nn_LmHeadTender).

fp8 (e5m2) DoubleRow implementation, v3.

Math (per core, vocab-sharded; vocab shard = 4000 rows, no padding):
    Wl   = dequant_int4(lm_weight)          # per-row scale sw = rowmax/7
    y    = dequant_int4(x, per-(chunk,channel) scale s = tmax*2^(b-13)/7)
    out  = y @ Wl.T
Every scale is factored out of the matmul so both operands are exactly
representable in fp8 e5m2:
    qw  in [-7, 7]             (weight ints; |w/s| <= 7 by construction)
    yq  = qx * 2^(bucket-13)   (activation ints scaled by a power of 2)
    out[t, v] = (tmax_c/7) * sw[v] * sum_h yq[t, h] * qw[v, h]
The rank-1 dequant scale (tmax_c/7) * sw[v] is applied on the host
during the unshard/gather; the device produces the raw fp8 matmul
accumulations (bf16).  All quantization (stats, buckets, rounding, for
both weights and activations) happens on device.

Measured hardware facts this version is built around:
  * A DR matmul streams 1 output column/cycle (500-col matmul = 208 ns);
    the 4096 main matmuls are a hard ~853 us floor per core.  LDWEIGHTS
    overlaps the previous matmul, so stationary reloads are free.
  * The matmul rhs must be contiguous along the streamed (vocab) dim —
    a strided rhs runs ~5x slower.  Hence qw lives as [128, kt, v].
  * The weight transpose into that layout is PE work (~110 us) and is
    scheduled inside the ~215 us weight-DMA window where the PE would
    otherwise idle: transposes get 2 dedicated PSUM banks, the matmuls
    use the other 6 (two 4-bank vocab passes per token half), so the
    first chunks' matmuls overlap the rest of the weight phase.
  * Activation quantization is 2 fused DVE passes per half-chunk using
    a per-channel magic constant Mp = 1.5*2^23 * 2^(bucket-13):
        t = x*(7/tmax) + Mp ;  y = t - Mp     (round-at-bit trick)
    with bucket derived from exponent bit arithmetic.
"""

import numpy as np
from contextlib import ExitStack

import concourse.bass as bass
import concourse.tile as tile
from concourse import bacc, masks, mybir
from concourse.bass_utils import run_bass_kernel_spmd

FP = mybir.dt.float32
BF = mybir.dt.bfloat16
F8 = mybir.dt.float8e5
I32 = mybir.dt.int32
ALU = mybir.AluOpType
AX = mybir.AxisListType
ACT = mybir.ActivationFunctionType
DR = mybir.MatmulPerfMode.DoubleRow

T = 4096            # tokens (2*2048)
H = 4096            # hidden
V = 32000           # vocab
NCORE = 8
VSH = V // NCORE    # 4000 vocab rows per core
CHUNK = 256
NCHUNK = T // CHUNK  # 16
KT = H // 128       # 32 k tiles (h = kt*128 + p)
KP = KT // 2        # 16 k pairs (DoubleRow)
VBS = 500           # vocab block size (one PSUM bank holds 512 fp32)
VB = VSH // VBS     # 8 blocks
MT = 32             # weight row tiles: 31 x 128 + 1 x 32
HHALF = H // 2      # weight h-half (2048)
KTH = KT // 2       # k tiles per h-half (16)
QMAX = 7.0
C_MAGIC = 12582912.0   # 1.5 * 2^23: round-to-nearest-even via add/sub
C7 = float(np.float32(1.0) / np.float32(7.0))  # fl(1/7); no DVE divide
# Mp bits = ((e2 + 23) << 23) | (1 << 22) = (e2 << 23) + MP_ADD
MP_ADD = (23 << 23) + (1 << 22)


def _emit(ctx: ExitStack, tc: "tile.TileContext", x_d, w_d, out_d):
    nc = tc.nc

    # ---------------- persistent tiles ----------------
    cpool = ctx.enter_context(tc.tile_pool(name="consts", bufs=1))
    ident_bf = cpool.tile([128, 128], BF)
    masks.make_identity(nc, ident_bf[:])
    qw_sb = cpool.tile([128, KT, VSH], F8)  # quantized weight^T, resident

    ypool = ctx.enter_context(tc.tile_pool(name="yq", bufs=4))
    stpool = ctx.enter_context(tc.tile_pool(name="xst", bufs=2))
    stgpool = ctx.enter_context(tc.tile_pool(name="stg", bufs=2))
    mpspool = ctx.enter_context(
        tc.tile_pool(name="mps", bufs=6, space="PSUM"))
    # phase-scoped pools (weight phase vs main loop) are opened in the
    # emission tail so their SBUF regions can be reused across phases.
    pools = {}

    # ---------------- weight tile quantization ----------------
    def emit_w_tile(m):
        rows = 128 if m < MT - 1 else VSH - 128 * (MT - 1)  # 32 for last
        halves = []
        for hh in range(2):
            w_nat = pools['wq'].tile([128, HHALF], FP, tag="w_nat",
                               name=f"w_nat_{m}_{hh}")
            # split each half across two queues to halve its latency
            for qd in range(2):
                w_dma = (nc.sync, nc.scalar, nc.gpsimd,
                         nc.sync)[hh * 2 + qd]
                hq = HHALF // 2
                w_dma.dma_start(
                    w_nat[:rows, qd * hq:(qd + 1) * hq],
                    w_d[m * 128:m * 128 + rows,
                        hh * HHALF + qd * hq:hh * HHALF + (qd + 1) * hq])
            rmax = pools['wst'].tile([128, 1], FP, tag="rmax",
                               name=f"rmax_{m}_{hh}")
            nc.vector.tensor_reduce(
                rmax[:rows], w_nat[:rows], axis=AX.X, op=ALU.max,
                apply_absolute_value=True)
            halves.append((w_nat, rmax))
        sw = pools['wst'].tile([128, 1], FP, tag="sw", name=f"sw_{m}")
        nc.vector.tensor_tensor(
            sw[:rows], halves[0][1][:rows], halves[1][1][:rows], op=ALU.max)
        # sw = max(rowmax*(1/7), 1e-9)  (reference: max(rowmax/7, 1e-9))
        nc.vector.tensor_scalar(
            sw[:rows], sw[:rows], C7, 1e-9, ALU.mult, ALU.max)
        rw = pools['wst'].tile([128, 1], FP, tag="rw", name=f"rw_{m}")
        nc.vector.reciprocal(rw[:rows], sw[:rows])
        for hh in range(2):
            w_nat = halves[hh][0]
            # round(w*rw): |w*rw| <= 7 so no clamp needed
            nc.scalar.activation(
                w_nat[:rows], w_nat[:rows], ACT.Copy,
                bias=C_MAGIC, scale=rw[:rows])
            for g in range(2):
                qi = pools['qi'].tile([128, 8, 128], BF, tag="qi",
                                 name=f"qi_{m}_{hh}_{g}")
                src = w_nat[:rows, g * 1024:(g + 1) * 1024].rearrange(
                    "v (q t) -> v q t", q=8)
                if (hh + g) % 2 == 0:
                    nc.vector.tensor_scalar(
                        qi[:rows], src, C_MAGIC, None, ALU.subtract)
                else:
                    nc.scalar.activation(
                        qi[:rows], src, ACT.Copy, bias=-C_MAGIC)
                ps = pools['wtp'].tile([128, 8, 128], BF, tag="wtp",
                                 name=f"wtp_{m}_{hh}_{g}")
                for qq in range(8):
                    nc.tensor.transpose(
                        ps[:, qq, 0:rows],
                        qi[:rows, qq, :],
                        ident_bf[:rows, :rows])
                kbase = hh * KTH + g * 8
                dst = qw_sb[:, kbase:kbase + 8, m * 128:m * 128 + rows]
                if (hh + g) % 2 == 0:
                    nc.scalar.activation(dst, ps[:, :, 0:rows], ACT.Copy)
                else:
                    nc.vector.tensor_copy(dst, ps[:, :, 0:rows])

    # ---------------- activation stats + quantization ----------------
    y_tiles = {}

    def emit_x(c):
        y_c = ypool.tile([128, KT, CHUNK], F8, tag="y", name=f"y_{c}")
        y_tiles[c] = y_c
        xhs = []
        cmaxs = []
        for th in range(2):
            xT = pools['xT'].tile([128, KT, 128], FP, tag="xT",
                            name=f"xT_{c}_{th}")
            src = x_d[:, c * CHUNK + th * 128:c * CHUNK + (th + 1) * 128]
            nc.gpsimd.dma_start(
                xT[:], src.rearrange("(k p) t -> p k t", p=128))
            cmh = stpool.tile([128, KT], FP, tag="cmh",
                              name=f"cmh_{c}_{th}")
            nc.vector.tensor_reduce(
                cmh[:], xT[:], axis=AX.X, op=ALU.max,
                apply_absolute_value=True)
            xhs.append(xT)
            cmaxs.append(cmh)
        cmax = stpool.tile([128, KT], FP, tag="cmax", name=f"cmax_{c}")
        nc.vector.tensor_tensor(cmax[:], cmaxs[0][:], cmaxs[1][:],
                                op=ALU.max)
        # ---- tmax: reduce cmax across free dim, then across partitions
        tpad = stpool.tile([128, 32], FP, tag="tpad", name=f"tpad_{c}")
        nc.vector.memset(tpad[:], 0.0)
        nc.vector.tensor_reduce(
            tpad[:, 0:1], cmax[:], axis=AX.X, op=ALU.max)
        tt = stpool.tile([32, 128], FP, tag="tt", name=f"tt_{c}")
        for a in range(4):
            nc.vector.transpose(
                tt[:, a * 32:(a + 1) * 32], tpad[a * 32:(a + 1) * 32, :])
        tmax_sc = stpool.tile([1, 1], FP, tag="tmax_sc", name=f"tms_{c}")
        nc.vector.tensor_reduce(
            tmax_sc[:], tt[0:1, :], axis=AX.X, op=ALU.max)
        tmax_b = stpool.tile([128, 1], FP, tag="tmax_b", name=f"tmb_{c}")
        nc.gpsimd.partition_broadcast(tmax_b[:], tmax_sc[:])
        rt = stpool.tile([128, 1], FP, tag="rt", name=f"rt_{c}")
        nc.vector.reciprocal(rt[:], tmax_b[:])
        r7 = stpool.tile([128, 1], FP, tag="r7", name=f"r7_{c}")
        nc.vector.tensor_scalar(r7[:], rt[:], 7.0, None, ALU.mult)
        # ---- bucket via exponent arithmetic: z = cmax/tmax in (0,1];
        # e2 = biased exponent of z rounded UP to a power of two, clamped
        # to [114,127] (= 2^-13..2^0).  Mp = 1.5*2^23 * 2^(e2-127).
        z = stpool.tile([128, KT], FP, tag="z", name=f"z_{c}")
        nc.vector.tensor_scalar(z[:], cmax[:], rt[:], None, ALU.mult)
        e2 = stpool.tile([128, KT], I32, tag="e2", name=f"e2_{c}")
        nc.vector.tensor_scalar(
            e2[:], z[:].bitcast(I32), 0x7FFFFF, None, ALU.add)
        nc.vector.tensor_scalar(
            e2[:], e2[:], 23, None, ALU.logical_shift_right)
        nc.vector.tensor_scalar(e2[:], e2[:], 114, 127, ALU.max, ALU.min)
        mp = stpool.tile([128, KT], I32, tag="mp", name=f"mp_{c}")
        nc.vector.tensor_scalar(
            mp[:], e2[:], 23, None, ALU.logical_shift_left)
        nc.vector.tensor_scalar(
            mp[:], mp[:], int(MP_ADD), None, ALU.add)
        # ---- fused quantize: t = x*R + Mp (one DVE op per half);
        # y = t - Mp (per-q ops split between scalar and vector)
        mn = stpool.tile([128, KT], FP, tag="mn", name=f"mn_{c}")
        nc.vector.tensor_scalar(
            mn[:], mp[:].bitcast(FP), -1.0, None, ALU.mult)
        mn_f = mn[:]
        mp_f = mp[:].bitcast(FP)
        for th in range(2):
            xT = xhs[th]
            mp_bc = mp_f.rearrange(
                "p (k o) -> p k o", o=1).broadcast_to([128, KT, 128])
            nc.vector.scalar_tensor_tensor(
                xT[:], xT[:], r7[:], mp_bc, op0=ALU.mult, op1=ALU.add)
            for q in range(KT):
                dst = y_c[:, q, th * 128:(th + 1) * 128]
                if q % 4 < 3:
                    nc.scalar.activation(
                        dst, xT[:, q, :], ACT.Identity,
                        bias=mn_f[:, q:q + 1], scale=1.0)
                else:
                    nc.vector.tensor_scalar(
                        dst, xT[:, q, :], mp_f[:, q:q + 1], None,
                        ALU.subtract)

    # ---------------- matmuls + raw drain ----------------
    def emit_m(c):
        y_c = y_tiles.pop(c)
        # one psum bank per (th, vb) accumulation chain: the matmuls of a
        # chain depend only on y(c) and qw block vb, so the scheduler can
        # start block vb's chains as soon as its weight tiles land.
        for th in range(2):
            tt_idx = c * 2 + th
            for vb in range(VB):
                ps = mpspool.tile([128, 512], FP, tag="mps",
                                  name=f"mps_{c}_{th}_{vb}")
                for kp in range(KP):
                    nc.tensor.matmul(
                        ps[:, 0:VBS],
                        y_c[:, 2 * kp:2 * kp + 2,
                            th * 128:(th + 1) * 128],
                        qw_sb[:, 2 * kp:2 * kp + 2,
                              vb * VBS:(vb + 1) * VBS],
                        start=(kp == 0), stop=(kp == KP - 1),
                        perf_mode=DR)
                stg = stgpool.tile([128, VBS], BF, tag="stg",
                                   name=f"stg_{c}_{th}_{vb}")
                if vb % 4 == 3:
                    nc.vector.tensor_copy(stg[:], ps[:, 0:VBS])
                else:
                    nc.scalar.activation(stg[:], ps[:, 0:VBS], ACT.Copy)
                out_eng = nc.sync if vb % 2 == 0 else nc.scalar
                out_eng.dma_start(
                    out_d[tt_idx * 128:(tt_idx + 1) * 128,
                          vb * VBS:(vb + 1) * VBS],
                    stg[:])

    # ---------------- emission schedule (two phases) ----------------
    # Phase A: deep-pipelined weight quantization (owns most SBUF).
    with (
        tc.tile_pool(name="wq", bufs=5) as wpool_,
        tc.tile_pool(name="wst", bufs=2) as wspool_,
        tc.tile_pool(name="qi", bufs=2) as qipool_,
        tc.tile_pool(name="wtp", bufs=2, space="PSUM") as wtpool_,
    ):
        pools['wq'] = wpool_
        pools['wst'] = wspool_
        pools['qi'] = qipool_
        pools['wtp'] = wtpool_
        for m in range(MT):
            emit_w_tile(m)

    # Phase B: activation pipeline + matmuls (reuses phase A's SBUF).
    with tc.tile_pool(name="xT", bufs=2) as xpool_:
        pools['xT'] = xpool_
        for c in range(3):
            emit_x(c)
        for c in range(NCHUNK):
            if c + 3 < NCHUNK:
                emit_x(c + 3)
            emit_m(c)


def _ldw_sig(inst):
    ap = inst.ins[0]
    return (ap.memref, ap.offset, str(ap.ap), str(ap.dtype),
            str(inst.perf_mode), inst.is_transpose,
            str(inst.tile_position), str(inst.tile_size))


def _dedup_ldweights(nc):
    """Drop InstLdweights whose weights are already resident in the PE
    array (identical AP/mode as the previous load, only non-self-loading
    matmuls in between).  Only wait-free, update-free loads are removed,
    so no semaphore surgery is needed."""
    removed = 0
    for bb in nc.main_func.blocks:
        insts = bb.instructions
        cur = None
        keep = []
        for inst in insts:
            if inst.engine != mybir.EngineType.PE:
                keep.append(inst)
                continue
            if isinstance(inst, mybir.InstLdweights):
                si = inst.sync_info
                clean = si is None or (not si.on_wait and not si.on_update)
                sig = _ldw_sig(inst)
                if clean and sig == cur:
                    removed += 1
                    continue
                cur = sig
                keep.append(inst)
            else:
                if not (isinstance(inst, mybir.InstMatmult)
                        and inst.ldweights is False):
                    cur = None  # self-loading matmul or other PE op
                keep.append(inst)
        if removed:
            insts[:] = keep
    return removed


_CACHED = None


def _build():
    global _CACHED
    if _CACHED is not None:
        return _CACHED
    nc = bacc.Bacc(
        "TRN2", target_bir_lowering=False, debug=False,
        enable_asserts=False, num_devices=NCORE)
    x_d = nc.dram_tensor("x", (H, T), FP, kind="ExternalInput").ap()
    w_d = nc.dram_tensor("w", (VSH, H), FP, kind="ExternalInput").ap()
    out_d = nc.dram_tensor("out", (T, VSH), BF, kind="ExternalOutput").ap()
    with tile.TileContext(nc) as tc:
        with ExitStack() as ctx:
            _emit(ctx, tc, x_d, w_d, out_d)
    _dedup_ldweights(nc)
    nc.compile()
    _CACHED = nc
    return nc


def kernel(hidden_states: np.ndarray, lm_weight: np.ndarray) -> np.ndarray:
    b, t, h = hidden_states.shape
    assert (b * t, h) == (T, H) and lm_weight.shape == (V, H)
    x2 = hidden_states.reshape(T, H).astype(np.float32)
    x_full = np.ascontiguousarray(x2.T)
    in_maps = []
    for c in range(NCORE):
        shard = np.ascontiguousarray(
            lm_weight[c * VSH:(c + 1) * VSH].astype(np.float32))
        in_maps.append({"x": x_full, "w": shard})
    nc = _build()
    res = run_bass_kernel_spmd(nc, in_maps, core_ids=list(range(NCORE)))
    outs = [np.asarray(res.results[c]["out"]).astype(np.float32)
            for c in range(NCORE)]
    full = np.concatenate(outs, axis=1)  # [T, V] raw accumulations
    # rank-1 dequant epilogue: out[t, v] = raw * (tmax_chunk(t)/7) * sw[v]
    # (identical formulas to the on-device quantization scales)
    w32 = lm_weight.astype(np.float32)
    sw = np.maximum(np.abs(w32).max(axis=1) * np.float32(C7),
                    np.float32(1e-9)).astype(np.float32)      # [V]
    tmax = np.abs(x2.reshape(NCHUNK, CHUNK * H)).max(axis=1)  # [16]
    m7 = (tmax * np.float32(C7)).astype(np.float32)
    full *= np.repeat(m7, CHUNK)[:, None]
    full *= sw[None, :]
    return full.reshape(b, t, V)


# revision 46
# speedup vs baseline: 1.0003x; 1.0003x over previous
"""Trainium2 Bass kernel for the quantized LM-head (nn_LmHeadTender).

fp8 (e5m2) DoubleRow implementation, v3.

Math (per core, vocab-sharded; vocab shard = 4000 rows, no padding):
    Wl   = dequant_int4(lm_weight)          # per-row scale sw = rowmax/7
    y    = dequant_int4(x, per-(chunk,channel) scale s = tmax*2^(b-13)/7)
    out  = y @ Wl.T
Every scale is factored out of the matmul so both operands are exactly
representable in fp8 e5m2:
    qw  in [-7, 7]             (weight ints; |w/s| <= 7 by construction)
    yq  = qx * 2^(bucket-13)   (activation ints scaled by a power of 2)
    out[t, v] = (tmax_c/7) * sw[v] * sum_h yq[t, h] * qw[v, h]
The rank-1 dequant scale (tmax_c/7) * sw[v] is applied on the host
during the unshard/gather; the device produces the raw fp8 matmul
accumulations (bf16).  All quantization (stats, buckets, rounding, for
both weights and activations) happens on device.

Measured hardware facts this version is built around:
  * A DR matmul streams 1 output column/cycle (500-col matmul = 208 ns);
    the 4096 main matmuls are a hard ~853 us floor per core.  LDWEIGHTS
    overlaps the previous matmul, so stationary reloads are free.
  * The matmul rhs must be contiguous along the streamed (vocab) dim —
    a strided rhs runs ~5x slower.  Hence qw lives as [128, kt, v].
  * The weight transpose into that layout is PE work (~110 us) and is
    scheduled inside the ~215 us weight-DMA window where the PE would
    otherwise idle: transposes get 2 dedicated PSUM banks, the matmuls
    use the other 6 (two 4-bank vocab passes per token half), so the
    first chunks' matmuls overlap the rest of the weight phase.
  * Activation quantization is 2 fused DVE passes per half-chunk using
    a per-channel magic constant Mp = 1.5*2^23 * 2^(bucket-13):
        t = x*(7/tmax) + Mp ;  y = t - Mp     (round-at-bit trick)
    with bucket derived from exponent bit arithmetic.
"""

import numpy as np
from contextlib import ExitStack

import concourse.bass as bass
import concourse.tile as tile
from concourse import bacc, masks, mybir
from concourse.bass_utils import run_bass_kernel_spmd

FP = mybir.dt.float32
BF = mybir.dt.bfloat16
F8 = mybir.dt.float8e5
I32 = mybir.dt.int32
ALU = mybir.AluOpType
AX = mybir.AxisListType
ACT = mybir.ActivationFunctionType
DR = mybir.MatmulPerfMode.DoubleRow

T = 4096            # tokens (2*2048)
H = 4096            # hidden
V = 32000           # vocab
NCORE = 8
VSH = V // NCORE    # 4000 vocab rows per core
CHUNK = 256
NCHUNK = T // CHUNK  # 16
KT = H // 128       # 32 k tiles (h = kt*128 + p)
KP = KT // 2        # 16 k pairs (DoubleRow)
VBS = 500           # vocab block size (one PSUM bank holds 512 fp32)
VB = VSH // VBS     # 8 blocks
MT = 32             # weight row tiles: 31 x 128 + 1 x 32
HHALF = H // 2      # weight h-half (2048)
KTH = KT // 2       # k tiles per h-half (16)
QMAX = 7.0
C_MAGIC = 12582912.0   # 1.5 * 2^23: round-to-nearest-even via add/sub
C7 = float(np.float32(1.0) / np.float32(7.0))  # fl(1/7); no DVE divide
# Mp bits = ((e2 + 23) << 23) | (1 << 22) = (e2 << 23) + MP_ADD
MP_ADD = (23 << 23) + (1 << 22)


def _emit(ctx: ExitStack, tc: "tile.TileContext", x_d, w_d, out_d):
    nc = tc.nc

    # ---------------- persistent tiles ----------------
    cpool = ctx.enter_context(tc.tile_pool(name="consts", bufs=1))
    ident_bf = cpool.tile([128, 128], BF)
    masks.make_identity(nc, ident_bf[:])
    qw_sb = cpool.tile([128, KT, VSH], F8)  # quantized weight^T, resident

    ypool = ctx.enter_context(tc.tile_pool(name="yq", bufs=4))
    stpool = ctx.enter_context(tc.tile_pool(name="xst", bufs=2))
    stgpool = ctx.enter_context(tc.tile_pool(name="stg", bufs=2))
    # phase-scoped pools (weight phase vs main loop) are opened in the
    # emission tail so their SBUF regions can be reused across phases.
    pools = {}

    # ---------------- weight tile quantization ----------------
    def emit_w_tile(m):
        rows = 128 if m < MT - 1 else VSH - 128 * (MT - 1)  # 32 for last
        halves = []
        for hh in range(2):
            w_nat = pools['wq'].tile([128, HHALF], FP, tag="w_nat",
                               name=f"w_nat_{m}_{hh}")
            w_dma = nc.sync if hh == 0 else nc.scalar
            w_dma.dma_start(
                w_nat[:rows], w_d[m * 128:m * 128 + rows,
                                  hh * HHALF:(hh + 1) * HHALF])
            rmax = pools['wst'].tile([128, 1], FP, tag="rmax",
                               name=f"rmax_{m}_{hh}")
            nc.vector.tensor_reduce(
                rmax[:rows], w_nat[:rows], axis=AX.X, op=ALU.max,
                apply_absolute_value=True)
            halves.append((w_nat, rmax))
        sw = pools['wst'].tile([128, 1], FP, tag="sw", name=f"sw_{m}")
        nc.vector.tensor_tensor(
            sw[:rows], halves[0][1][:rows], halves[1][1][:rows], op=ALU.max)
        # sw = max(rowmax*(1/7), 1e-9)  (reference: max(rowmax/7, 1e-9))
        nc.vector.tensor_scalar(
            sw[:rows], sw[:rows], C7, 1e-9, ALU.mult, ALU.max)
        rw = pools['wst'].tile([128, 1], FP, tag="rw", name=f"rw_{m}")
        nc.vector.reciprocal(rw[:rows], sw[:rows])
        for hh in range(2):
            w_nat = halves[hh][0]
            # round(w*rw): |w*rw| <= 7 so no clamp needed
            nc.scalar.activation(
                w_nat[:rows], w_nat[:rows], ACT.Copy,
                bias=C_MAGIC, scale=rw[:rows])
            for g in range(2):
                qi = pools['qi'].tile([128, 8, 128], BF, tag="qi",
                                 name=f"qi_{m}_{hh}_{g}")
                src = w_nat[:rows, g * 1024:(g + 1) * 1024].rearrange(
                    "v (q t) -> v q t", q=8)
                if (hh + g) % 2 == 0:
                    nc.vector.tensor_scalar(
                        qi[:rows], src, C_MAGIC, None, ALU.subtract)
                else:
                    nc.scalar.activation(
                        qi[:rows], src, ACT.Copy, bias=-C_MAGIC)
                ps = pools['wtp'].tile([128, 8, 128], BF, tag="wtp",
                                 name=f"wtp_{m}_{hh}_{g}")
                for qq in range(8):
                    nc.tensor.transpose(
                        ps[:, qq, 0:rows],
                        qi[:rows, qq, :],
                        ident_bf[:rows, :rows])
                kbase = hh * KTH + g * 8
                dst = qw_sb[:, kbase:kbase + 8, m * 128:m * 128 + rows]
                if (hh + g) % 2 == 0:
                    nc.scalar.activation(dst, ps[:, :, 0:rows], ACT.Copy)
                else:
                    nc.vector.tensor_copy(dst, ps[:, :, 0:rows])

    # ---------------- activation stats + quantization ----------------
    y_tiles = {}

    def emit_x(c):
        y_c = ypool.tile([128, KT, CHUNK], F8, tag="y", name=f"y_{c}")
        y_tiles[c] = y_c
        xhs = []
        cmaxs = []
        for th in range(2):
            xT = pools['xT'].tile([128, KT, 128], FP, tag="xT",
                            name=f"xT_{c}_{th}")
            src = x_d[:, c * CHUNK + th * 128:c * CHUNK + (th + 1) * 128]
            nc.gpsimd.dma_start(
                xT[:], src.rearrange("(k p) t -> p k t", p=128))
            cmh = stpool.tile([128, KT], FP, tag="cmh",
                              name=f"cmh_{c}_{th}")
            nc.vector.tensor_reduce(
                cmh[:], xT[:], axis=AX.X, op=ALU.max,
                apply_absolute_value=True)
            xhs.append(xT)
            cmaxs.append(cmh)
        cmax = stpool.tile([128, KT], FP, tag="cmax", name=f"cmax_{c}")
        nc.vector.tensor_tensor(cmax[:], cmaxs[0][:], cmaxs[1][:],
                                op=ALU.max)
        # ---- tmax: reduce cmax across free dim, then across partitions
        tpad = stpool.tile([128, 32], FP, tag="tpad", name=f"tpad_{c}")
        nc.vector.memset(tpad[:], 0.0)
        nc.vector.tensor_reduce(
            tpad[:, 0:1], cmax[:], axis=AX.X, op=ALU.max)
        tt = stpool.tile([32, 128], FP, tag="tt", name=f"tt_{c}")
        for a in range(4):
            nc.vector.transpose(
                tt[:, a * 32:(a + 1) * 32], tpad[a * 32:(a + 1) * 32, :])
        tmax_sc = stpool.tile([1, 1], FP, tag="tmax_sc", name=f"tms_{c}")
        nc.vector.tensor_reduce(
            tmax_sc[:], tt[0:1, :], axis=AX.X, op=ALU.max)
        tmax_b = stpool.tile([128, 1], FP, tag="tmax_b", name=f"tmb_{c}")
        nc.gpsimd.partition_broadcast(tmax_b[:], tmax_sc[:])
        rt = stpool.tile([128, 1], FP, tag="rt", name=f"rt_{c}")
        nc.vector.reciprocal(rt[:], tmax_b[:])
        r7 = stpool.tile([128, 1], FP, tag="r7", name=f"r7_{c}")
        nc.vector.tensor_scalar(r7[:], rt[:], 7.0, None, ALU.mult)
        # ---- bucket via exponent arithmetic: z = cmax/tmax in (0,1];
        # e2 = biased exponent of z rounded UP to a power of two, clamped
        # to [114,127] (= 2^-13..2^0).  Mp = 1.5*2^23 * 2^(e2-127).
        z = stpool.tile([128, KT], FP, tag="z", name=f"z_{c}")
        nc.vector.tensor_scalar(z[:], cmax[:], rt[:], None, ALU.mult)
        e2 = stpool.tile([128, KT], I32, tag="e2", name=f"e2_{c}")
        nc.vector.tensor_scalar(
            e2[:], z[:].bitcast(I32), 0x7FFFFF, None, ALU.add)
        nc.vector.tensor_scalar(
            e2[:], e2[:], 23, None, ALU.logical_shift_right)
        nc.vector.tensor_scalar(e2[:], e2[:], 114, 127, ALU.max, ALU.min)
        mp = stpool.tile([128, KT], I32, tag="mp", name=f"mp_{c}")
        nc.vector.tensor_scalar(
            mp[:], e2[:], 23, None, ALU.logical_shift_left)
        nc.vector.tensor_scalar(
            mp[:], mp[:], int(MP_ADD), None, ALU.add)
        # ---- fused quantize: t = x*R + Mp (one DVE op per half);
        # y = t - Mp (per-q ops split between scalar and vector)
        mn = stpool.tile([128, KT], FP, tag="mn", name=f"mn_{c}")
        nc.vector.tensor_scalar(
            mn[:], mp[:].bitcast(FP), -1.0, None, ALU.mult)
        mn_f = mn[:]
        mp_f = mp[:].bitcast(FP)
        for th in range(2):
            xT = xhs[th]
            mp_bc = mp_f.rearrange(
                "p (k o) -> p k o", o=1).broadcast_to([128, KT, 128])
            nc.vector.scalar_tensor_tensor(
                xT[:], xT[:], r7[:], mp_bc, op0=ALU.mult, op1=ALU.add)
            for q in range(KT):
                dst = y_c[:, q, th * 128:(th + 1) * 128]
                if q % 4 < 3:
                    nc.scalar.activation(
                        dst, xT[:, q, :], ACT.Identity,
                        bias=mn_f[:, q:q + 1], scale=1.0)
                else:
                    nc.vector.tensor_scalar(
                        dst, xT[:, q, :], mp_f[:, q:q + 1], None,
                        ALU.subtract)

    # ---------------- matmuls + raw drain ----------------
    def emit_m(c):
        y_c = y_tiles.pop(c)
        # one psum bank per (th, vb) accumulation chain: the matmuls of a
        # chain depend only on y(c) and qw block vb, so the scheduler can
        # start block vb's chains as soon as its weight tiles land.
        for th in range(2):
            tt_idx = c * 2 + th
            for vb in range(VB):
                ps = pools['mps'].tile([128, 512], FP, tag="mps",
                                  name=f"mps_{c}_{th}_{vb}")
                for kp in range(KP):
                    nc.tensor.matmul(
                        ps[:, 0:VBS],
                        y_c[:, 2 * kp:2 * kp + 2,
                            th * 128:(th + 1) * 128],
                        qw_sb[:, 2 * kp:2 * kp + 2,
                              vb * VBS:(vb + 1) * VBS],
                        start=(kp == 0), stop=(kp == KP - 1),
                        perf_mode=DR)
                stg = stgpool.tile([128, VBS], BF, tag="stg",
                                   name=f"stg_{c}_{th}_{vb}")
                if vb % 4 == 3:
                    nc.vector.tensor_copy(stg[:], ps[:, 0:VBS])
                else:
                    nc.scalar.activation(stg[:], ps[:, 0:VBS], ACT.Copy)
                out_eng = nc.sync if vb % 2 == 0 else nc.scalar
                out_eng.dma_start(
                    out_d[tt_idx * 128:(tt_idx + 1) * 128,
                          vb * VBS:(vb + 1) * VBS],
                    stg[:])

    # ---------------- emission schedule (two phases) ----------------
    # Phase A: deep-pipelined weight quantization (owns most SBUF).
    with (
        tc.tile_pool(name="wq", bufs=5) as wpool_,
        tc.tile_pool(name="wst", bufs=2) as wspool_,
        tc.tile_pool(name="qi", bufs=2) as qipool_,
        tc.tile_pool(name="wtp", bufs=2, space="PSUM") as wtpool_,
    ):
        pools['wq'] = wpool_
        pools['wst'] = wspool_
        pools['qi'] = qipool_
        pools['wtp'] = wtpool_
        for m in range(MT):
            emit_w_tile(m)

    # Phase B: activation pipeline + matmuls (reuses phase A's SBUF).
    with (
        tc.tile_pool(name="xT", bufs=2) as xpool_,
        tc.tile_pool(name="mps", bufs=8, space="PSUM") as mpspool_,
    ):
        pools['xT'] = xpool_
        pools['mps'] = mpspool_
        for c in range(3):
            emit_x(c)
        for c in range(NCHUNK):
            if c + 3 < NCHUNK:
                emit_x(c + 3)
            emit_m(c)


def _ldw_sig(inst):
    ap = inst.ins[0]
    return (ap.memref, ap.offset, str(ap.ap), str(ap.dtype),
            str(inst.perf_mode), inst.is_transpose,
            str(inst.tile_position), str(inst.tile_size))


def _dedup_ldweights(nc):
    """Drop InstLdweights whose weights are already resident in the PE
    array (identical AP/mode as the previous load, only non-self-loading
    matmuls in between).  Only wait-free, update-free loads are removed,
    so no semaphore surgery is needed."""
    removed = 0
    for bb in nc.main_func.blocks:
        insts = bb.instructions
        cur = None
        keep = []
        for inst in insts:
            if inst.engine != mybir.EngineType.PE:
                keep.append(inst)
                continue
            if isinstance(inst, mybir.InstLdweights):
                si = inst.sync_info
                clean = si is None or (not si.on_wait and not si.on_update)
                sig = _ldw_sig(inst)
                if clean and sig == cur:
                    removed += 1
                    continue
                cur = sig
                keep.append(inst)
            else:
                if not (isinstance(inst, mybir.InstMatmult)
                        and inst.ldweights is False):
                    cur = None  # self-loading matmul or other PE op
                keep.append(inst)
        if removed:
            insts[:] = keep
    return removed


_CACHED = None


def _build():
    global _CACHED
    if _CACHED is not None:
        return _CACHED
    nc = bacc.Bacc(
        "TRN2", target_bir_lowering=False, debug=False,
        enable_asserts=False, num_devices=NCORE)
    x_d = nc.dram_tensor("x", (H, T), FP, kind="ExternalInput").ap()
    w_d = nc.dram_tensor("w", (VSH, H), FP, kind="ExternalInput").ap()
    out_d = nc.dram_tensor("out", (T, VSH), BF, kind="ExternalOutput").ap()
    with tile.TileContext(nc) as tc:
        with ExitStack() as ctx:
            _emit(ctx, tc, x_d, w_d, out_d)
    _dedup_ldweights(nc)
    nc.compile()
    _CACHED = nc
    return nc


def kernel(hidden_states: np.ndarray, lm_weight: np.ndarray) -> np.ndarray:
    b, t, h = hidden_states.shape
    assert (b * t, h) == (T, H) and lm_weight.shape == (V, H)
    x2 = hidden_states.reshape(T, H).astype(np.float32)
    x_full = np.ascontiguousarray(x2.T)
    in_maps = []
    for c in range(NCORE):
        shard = np.ascontiguousarray(
            lm_weight[c * VSH:(c + 1) * VSH].astype(np.float32))
        in_maps.append({"x": x_full, "w": shard})
    nc = _build()
    res = run_bass_kernel_spmd(nc, in_maps, core_ids=list(range(NCORE)))
    outs = [np.asarray(res.results[c]["out"]).astype(np.float32)
            for c in range(NCORE)]
    full = np.concatenate(outs, axis=1)  # [T, V] raw accumulations
    # rank-1 dequant epilogue: out[t, v] = raw * (tmax_chunk(t)/7) * sw[v]
    # (identical formulas to the on-device quantization scales)
    w32 = lm_weight.astype(np.float32)
    sw = np.maximum(np.abs(w32).max(axis=1) * np.float32(C7),
                    np.float32(1e-9)).astype(np.float32)      # [V]
    tmax = np.abs(x2.reshape(NCHUNK, CHUNK * H)).max(axis=1)  # [16]
    m7 = (tmax * np.float32(C7)).astype(np.float32)
    full *= np.repeat(m7, CHUNK)[:, None]
    full *= sw[None, :]
    return full.reshape(b, t, V)
